# revision 10
# baseline (speedup 1.0000x reference)
import sys, os
sys.path.insert(0, "/opt/trn_rl_repo")
sys.path.insert(0, "/opt/trn_rl_repo/concourse")

import numpy as np
import ml_dtypes

import concourse.bass as bass
import concourse.bacc as bacc
import concourse.mybir as mybir
from concourse.tile import TileContext
from concourse.bass_utils import run_bass_kernel_spmd

BF16 = ml_dtypes.bfloat16

B, S, E, H = 8, 4096, 1024, 16
HD = E // H  # 64
SCALE = 1.0 / float(np.sqrt(HD))
TOK_TILE = 128
N_TILES = S // TOK_TILE  # 32
N_CHUNK = E // 128  # 8

f32 = mybir.dt.float32
bf16 = mybir.dt.bfloat16
AF = mybir.ActivationFunctionType
ALU = mybir.AluOpType
AX = mybir.AxisListType


def build_nc():
    nc = bacc.Bacc()
    # x arrives pre-permuted to head-blocked (n, w) layout and pre-cast to bf16
    x_d = nc.dram_tensor("x", [S, E], bf16, kind="ExternalInput")
    bdq_d = nc.dram_tensor("bdq", [128, 128], bf16, kind="ExternalInput")
    bdk_d = nc.dram_tensor("bdk", [128, 130], bf16, kind="ExternalInput")
    bdv_d = nc.dram_tensor("bdv", [128, 128], bf16, kind="ExternalInput")
    ident_d = nc.dram_tensor("ident", [128, 128], bf16, kind="ExternalInput")
    wot_d = nc.dram_tensor("wot", [E, E], bf16, kind="ExternalInput")
    ones_d = nc.dram_tensor("ones1", [1, 128], bf16, kind="ExternalInput")
    bo_d = nc.dram_tensor("bo_eff", [1, E], bf16, kind="ExternalInput")
    out_d = nc.dram_tensor("out", [S, E], f32, kind="ExternalOutput")

    with TileContext(nc) as tc:
        with (
            tc.tile_pool(name="const", bufs=1) as cpool,
            tc.tile_pool(name="work", bufs=2) as wpool,
            tc.tile_pool(name="big", bufs=2) as bpool,
            tc.tile_pool(name="mid", bufs=1) as mpool,
            tc.tile_pool(name="ps", bufs=2, space="PSUM") as tps_pool,
            tc.tile_pool(name="qkvps", bufs=3, space="PSUM") as qkv_pool,
            tc.tile_pool(name="ops", bufs=2, space="PSUM") as o_pool,
        ):
            bdq = cpool.tile_from(bdq_d[:, :])
            bdk = cpool.tile_from(bdk_d[:, :])
            bdv = cpool.tile_from(bdv_d[:, :])
            ident = cpool.tile_from(ident_d[:, :])
            ones1 = cpool.tile_from(ones_d[:, :])
            bo_sb = cpool.tile_from(bo_d[:, :])
            wot = [cpool.tile_from(wot_d[128 * c:128 * (c + 1), :], name=f"wot{c}")
                   for c in range(N_CHUNK)]

            def front_end(ti):
                """DMA + transposes + qkv projections for token tile ti.
                Emitted one tile ahead so PE/ACT prep overlaps the previous
                tile's DVE attention chain."""
                r0 = ti * TOK_TILE
                xbf = wpool.tile([128, E], bf16, tag="xbf")
                nc.gpsimd.dma_start(out=xbf[:, :], in_=x_d[r0:r0 + 128, :])

                xT = wpool.tile([128, E], bf16, tag="xT")
                for c in range(N_CHUNK):
                    tp = tps_pool.tile([128, 128], bf16, tag="tps")
                    nc.tensor.transpose(
                        tp[:, :], xbf[:, 128 * c:128 * (c + 1)], ident[:, :])
                    nc.scalar.activation(
                        xT[:, 128 * c:128 * (c + 1)], tp[:, :], AF.Copy)

                q_sb = wpool.tile([128, E], bf16, tag="q")
                k_sb = wpool.tile([128, E], bf16, tag="k")
                # v stored in (w, m) transposed layout: vt[t, w*16+m] = v[t,m,w]
                vt_sb = wpool.tile([128, E], bf16, tag="v")
                r_sb = wpool.tile([128, H], f32, tag="r")

                for c in range(N_CHUNK):
                    for (w_sb, bd) in ((q_sb, bdq), (k_sb, bdk), (vt_sb, bdv)):
                        pp = qkv_pool.tile([128, 130], f32, tag="qkvps")
                        ncols = 130 if bd is bdk else 128
                        nc.tensor.matmul(
                            pp[:, 0:ncols],
                            lhsT=xT[:, 128 * c:128 * (c + 1)],
                            rhs=bd[:, 0:ncols], start=True, stop=True)
                        if bd is bdv:
                            # psum cols are (m_local, w); scatter into (w, m)
                            pv = pp[:, 0:128].rearrange(
                                "t (m w) -> t w m", m=2)
                            dv = vt_sb[:, :].rearrange(
                                "t (w m) -> t w m", m=H)[:, :, 2 * c:2 * c + 2]
                            nc.scalar.activation(dv, pv, AF.Copy)
                        else:
                            nc.scalar.activation(
                                w_sb[:, 128 * c:128 * (c + 1)], pp[:, 0:128],
                                AF.Copy)
                        if bd is bdk:
                            nc.scalar.activation(
                                r_sb[:, 2 * c:2 * c + 2], pp[:, 128:130],
                                AF.Copy)
                return q_sb, k_sb, vt_sb, r_sb

            def attention(q_sb, k_sb, vt_sb, r_sb):
                """Per-token 16x16 head-mixing attention, all on DVE/ACT."""
                # logits: prod[t,(n,m,w)] = q[t,n,w]*k[t,m,w]  (q pre-scaled)
                prod = bpool.tile([128, H * H * HD], bf16, tag="prod")
                qv = q_sb[:, :].rearrange("t (n o w) -> t n o w", n=H, o=1) \
                    .broadcast_to((128, H, H, HD))
                kv = k_sb[:, :].rearrange("t (o m w) -> t o m w", o=1, m=H) \
                    .broadcast_to((128, H, H, HD))
                # the logits product runs on GpSimd: its inputs are ready a
                # full tile-period early (pipelined front-end), so it overlaps
                # the previous tile's DVE chain and frees 8.7us/tile of DVE
                p3 = prod[:, :].rearrange("t (n m w) -> t n m w", n=H, m=H)
                nc.gpsimd.tensor_mul(p3, qv, kv)

                pa1 = mpool.tile([128, H * H * HD // 2], bf16, tag="pa1")
                pv_ = prod[:, :].rearrange("t (nm w) -> t nm w", w=HD)
                nc.vector.tensor_add(
                    pa1[:, :].rearrange("t (nm w) -> t nm w", w=HD // 2),
                    pv_[:, :, 0:HD // 2], pv_[:, :, HD // 2:HD])
                pa2 = mpool.tile([128, H * H * HD // 4], bf16, tag="pa2")
                p1v = pa1[:, :].rearrange("t (nm w) -> t nm w", w=HD // 2)
                nc.vector.tensor_add(
                    pa2[:, :].rearrange("t (nm w) -> t nm w", w=HD // 4),
                    p1v[:, :, 0:HD // 4], p1v[:, :, HD // 4:HD // 2])
                pa3 = mpool.tile([128, H * H * HD // 8], bf16, tag="pa3")
                p2v = pa2[:, :].rearrange("t (nm w) -> t nm w", w=HD // 4)
                nc.vector.tensor_add(
                    pa3[:, :].rearrange("t (nm w) -> t nm w", w=HD // 8),
                    p2v[:, :, 0:HD // 8], p2v[:, :, HD // 8:HD // 4])
                pa4 = mpool.tile([128, H * H * HD // 16], bf16, tag="pa4")
                p3v = pa3[:, :].rearrange("t (nm w) -> t nm w", w=HD // 8)
                nc.vector.tensor_add(
                    pa4[:, :].rearrange("t (nm w) -> t nm w", w=HD // 16),
                    p3v[:, :, 0:HD // 16], p3v[:, :, HD // 16:HD // 8])
                pa5 = mpool.tile([128, H * H * HD // 32], bf16, tag="pa5")
                p4v = pa4[:, :].rearrange("t (nm w) -> t nm w", w=HD // 16)
                nc.vector.tensor_add(
                    pa5[:, :].rearrange("t (nm w) -> t nm w", w=HD // 32),
                    p4v[:, :, 0:HD // 32], p4v[:, :, HD // 32:HD // 16])
                L = wpool.tile([128, H * H], f32, tag="L")
                p5v = pa5[:, :].rearrange("t (nm w) -> t nm w", w=HD // 32)
                nc.vector.tensor_add(
                    L[:, :].rearrange("t (nm o) -> t nm o", o=1),
                    p5v[:, :, 0:1], p5v[:, :, 1:2])
                # add r[t,m] broadcast over n; no max-subtraction (|L|
                # is O(8) so f32 exp is safe)
                rv = r_sb[:, :].rearrange("t (o m) -> t o m", o=1) \
                    .broadcast_to((128, H, H))
                L2 = wpool.tile([128, H * H], f32, tag="L2")
                nc.vector.tensor_add(
                    L2[:, :].rearrange("t (n m) -> t n m", n=H),
                    L[:, :].rearrange("t (n m) -> t n m", n=H), rv)

                Ex = wpool.tile([128, H * H], f32, tag="Ex")
                nc.scalar.activation(Ex[:, :], L2[:, :], AF.Exp)

                den = wpool.tile([128, H], f32, tag="den")
                nc.vector.reduce_sum(
                    den[:, :].rearrange("t (n o) -> t n o", o=1),
                    Ex[:, :].rearrange("t (n m) -> t n m", n=H), axis=AX.X)
                dinv = wpool.tile([128, H], f32, tag="dinv")
                nc.vector.reciprocal(dinv[:, :], den[:, :])
                attn_t = wpool.tile([128, H * H], bf16, tag="attn")
                dv_ = dinv[:, :].rearrange("t (n o) -> t n o", o=1) \
                    .broadcast_to((128, H, H))
                nc.vector.tensor_mul(
                    attn_t[:, :].rearrange("t (n m) -> t n m", n=H),
                    Ex[:, :].rearrange("t (n m) -> t n m", n=H), dv_)

                # AV: prod2[t,(n,w,m)] = attn[t,n,m]*vt[t,w,m]; reduce over m.
                # Both operands innermost-contiguous (m) -> DVE 2x mode.
                prod2 = bpool.tile([128, H * HD * H], bf16, tag="prod")
                av = attn_t[:, :].rearrange("t (n o m) -> t n o m", n=H, o=1) \
                    .broadcast_to((128, H, HD, H))
                vv = vt_sb[:, :].rearrange("t (o w m) -> t o w m", o=1, m=H) \
                    .broadcast_to((128, H, HD, H))
                nc.vector.tensor_mul(
                    prod2[:, :].rearrange("t (n w m) -> t n w m", n=H, w=HD),
                    av, vv)
                qa1 = mpool.tile([128, E * H // 2], bf16, tag="pa1")
                q2v = prod2[:, :].rearrange("t (nw m) -> t nw m", m=H)
                nc.vector.tensor_add(
                    qa1[:, :].rearrange("t (nw m) -> t nw m", m=H // 2),
                    q2v[:, :, 0:H // 2], q2v[:, :, H // 2:H])
                qa2 = mpool.tile([128, E * H // 4], bf16, tag="qa2")
                q1v = qa1[:, :].rearrange("t (nw m) -> t nw m", m=H // 2)
                nc.vector.tensor_add(
                    qa2[:, :].rearrange("t (nw m) -> t nw m", m=H // 4),
                    q1v[:, :, 0:H // 4], q1v[:, :, H // 4:H // 2])
                qa3 = mpool.tile([128, E * H // 8], bf16, tag="qa3")
                q2v_ = qa2[:, :].rearrange("t (nw m) -> t nw m", m=H // 4)
                nc.vector.tensor_add(
                    qa3[:, :].rearrange("t (nw m) -> t nw m", m=H // 8),
                    q2v_[:, :, 0:H // 8], q2v_[:, :, H // 8:H // 4])
                vals_bf = wpool.tile([128, E], bf16, tag="valsbf")
                q3v = qa3[:, :].rearrange("t (nw m) -> t nw m", m=H // 8)
                nc.vector.tensor_add(
                    vals_bf[:, :].rearrange("t (nw o) -> t nw o", o=1),
                    q3v[:, :, 0:1], q3v[:, :, 1:2])
                return vals_bf

            def back_end(ti, vals_bf):
                r0 = ti * TOK_TILE
                vT = wpool.tile([128, E], bf16, tag="vT")
                for c in range(N_CHUNK):
                    tp = tps_pool.tile([128, 128], bf16, tag="tps")
                    nc.tensor.transpose(
                        tp[:, :], vals_bf[:, 128 * c:128 * (c + 1)], ident[:, :])
                    nc.scalar.activation(
                        vT[:, 128 * c:128 * (c + 1)], tp[:, :], AF.Copy)

                out_sb = wpool.tile([128, E], f32, tag="outsb")
                for half in range(2):
                    op = o_pool.tile([128, 512], f32, tag="ops")
                    for c in range(N_CHUNK):
                        nc.tensor.matmul(
                            op[:, :],
                            lhsT=vT[:, 128 * c:128 * (c + 1)],
                            rhs=wot[c][:, 512 * half:512 * (half + 1)],
                            start=(c == 0), stop=False)
                    nc.tensor.matmul(
                        op[:, :], lhsT=ones1[:, :],
                        rhs=bo_sb[:, 512 * half:512 * (half + 1)],
                        start=False, stop=True)
                    nc.scalar.activation(
                        out_sb[:, 512 * half:512 * (half + 1)], op[:, :], AF.Copy)
                nc.gpsimd.dma_start(out=out_d[r0:r0 + 128, :], in_=out_sb[:, :])

            # software pipeline: tile ti+1's front-end is emitted before
            # tile ti's attention so PE/ACT prep runs under the DVE chain
            cur = front_end(0)
            for ti in range(N_TILES):
                nxt = front_end(ti + 1) if ti + 1 < N_TILES else None
                vals_bf = attention(*cur)
                back_end(ti, vals_bf)
                cur = nxt
    nc.compile()
    return nc


def _host_consts(wq, bq, wk, bk, wv, bv, wo, bo):
    z = np.zeros((64, 64), np.float32)
    bd2 = lambda w: np.block([[w.T, z], [z, w.T]])
    bdq = (bd2(wq) * SCALE).astype(BF16)
    bdv = bd2(wv).astype(BF16)
    cr = (SCALE * (wk.T @ bq)).astype(np.float32)
    bdk = np.zeros((128, 130), np.float32)
    bdk[:, :128] = bd2(wk)
    bdk[0:64, 128] = cr
    bdk[64:128, 129] = cr
    bdk = bdk.astype(BF16)
    bo_eff = bo + wo @ np.tile(bv, H)
    return dict(
        bdq=bdq, bdk=bdk, bdv=bdv,
        ident=np.eye(128, dtype=BF16),
        wot=np.ascontiguousarray(wo.T).astype(BF16),
        ones1=np.ones((1, 128), BF16),
        bo_eff=bo_eff.reshape(1, E).astype(BF16),
    )


_CACHE = {}


def kernel(x, wq, bq, wk, bk, wv, bv, wo, bo, _trace=False):
    x = np.asarray(x, np.float32)
    # host-side permute e=(w n) -> head-blocked (n w), cast to bf16
    xp = np.ascontiguousarray(
        x.reshape(B, S, HD, H).transpose(0, 1, 3, 2)).reshape(B, S, E)
    xp = xp.astype(BF16)
    consts = _host_consts(
        np.asarray(wq, np.float32), np.asarray(bq, np.float32),
        np.asarray(wk, np.float32), np.asarray(bk, np.float32),
        np.asarray(wv, np.float32), np.asarray(bv, np.float32),
        np.asarray(wo, np.float32), np.asarray(bo, np.float32))
    if "nc" not in _CACHE:
        _CACHE["nc"] = build_nc()
    nc = _CACHE["nc"]
    in_maps = [dict(x=np.ascontiguousarray(xp[i]), **consts) for i in range(B)]
    res = run_bass_kernel_spmd(nc, in_maps, core_ids=list(range(B)), trace=_trace)
    out = np.stack([np.asarray(res.results[i]["out"]) for i in range(B)], axis=0)
    if _trace:
        kernel._last = res
    return out


# revision 12
# speedup vs baseline: 1.6479x; 1.6479x over previous
import sys, os
sys.path.insert(0, "/opt/trn_rl_repo")
sys.path.insert(0, "/opt/trn_rl_repo/concourse")

import numpy as np
import ml_dtypes

import concourse.bass as bass
import concourse.bacc as bacc
import concourse.mybir as mybir
from concourse.tile import TileContext
from concourse.bass_utils import run_bass_kernel_spmd

BF16 = ml_dtypes.bfloat16

B, S, E, H = 8, 4096, 1024, 16
HD = E // H  # 64
SCALE = 1.0 / float(np.sqrt(HD))
TOK_TILE = 128
N_TILES = S // TOK_TILE  # 32
N_CHUNK = E // 128  # 8

f32 = mybir.dt.float32
bf16 = mybir.dt.bfloat16
AF = mybir.ActivationFunctionType
ALU = mybir.AluOpType
AX = mybir.AxisListType


def build_nc():
    nc = bacc.Bacc()
    # x arrives pre-permuted to head-blocked (n, w) layout and pre-cast to bf16
    x_d = nc.dram_tensor("x", [S, E], bf16, kind="ExternalInput")
    bdq_d = nc.dram_tensor("bdq", [128, 128], bf16, kind="ExternalInput")
    bdk_d = nc.dram_tensor("bdk", [128, 130], bf16, kind="ExternalInput")
    bdv_d = nc.dram_tensor("bdv", [128, 128], bf16, kind="ExternalInput")
    ident_d = nc.dram_tensor("ident", [128, 128], bf16, kind="ExternalInput")
    wot_d = nc.dram_tensor("wot", [E, E], bf16, kind="ExternalInput")
    ones_d = nc.dram_tensor("ones1", [1, 128], bf16, kind="ExternalInput")
    bo_d = nc.dram_tensor("bo_eff", [1, E], bf16, kind="ExternalInput")
    out_d = nc.dram_tensor("out", [S, E], f32, kind="ExternalOutput")

    with TileContext(nc) as tc:
        with (
            tc.tile_pool(name="const", bufs=1) as cpool,
            tc.tile_pool(name="work", bufs=2) as wpool,
            tc.tile_pool(name="big", bufs=2) as bpool,
            tc.tile_pool(name="mid", bufs=1) as mpool,
            tc.tile_pool(name="ps", bufs=2, space="PSUM") as tps_pool,
            tc.tile_pool(name="qkvps", bufs=3, space="PSUM") as qkv_pool,
            tc.tile_pool(name="ops", bufs=2, space="PSUM") as o_pool,
        ):
            bdq = cpool.tile_from(bdq_d[:, :])
            bdk = cpool.tile_from(bdk_d[:, :])
            bdv = cpool.tile_from(bdv_d[:, :])
            ident = cpool.tile_from(ident_d[:, :])
            ones1 = cpool.tile_from(ones_d[:, :])
            bo_sb = cpool.tile_from(bo_d[:, :])
            wot = [cpool.tile_from(wot_d[128 * c:128 * (c + 1), :], name=f"wot{c}")
                   for c in range(N_CHUNK)]

            def front_end(ti):
                """DMA + transposes + qkv projections for token tile ti.
                Emitted one tile ahead so PE/ACT prep overlaps the previous
                tile's DVE attention chain."""
                r0 = ti * TOK_TILE
                xbf = wpool.tile([128, E], bf16, tag="xbf")
                nc.gpsimd.dma_start(out=xbf[:, :], in_=x_d[r0:r0 + 128, :])

                xT = wpool.tile([128, E], bf16, tag="xT")
                for c in range(N_CHUNK):
                    tp = tps_pool.tile([128, 128], bf16, tag="tps")
                    nc.tensor.transpose(
                        tp[:, :], xbf[:, 128 * c:128 * (c + 1)], ident[:, :])
                    nc.scalar.activation(
                        xT[:, 128 * c:128 * (c + 1)], tp[:, :], AF.Copy)

                q_sb = wpool.tile([128, E], bf16, tag="q")
                k_sb = wpool.tile([128, E], bf16, tag="k")
                # v stored in (w, m) transposed layout: vt[t, w*16+m] = v[t,m,w]
                vt_sb = wpool.tile([128, E], bf16, tag="v")
                r_sb = wpool.tile([128, H], f32, tag="r")

                for c in range(N_CHUNK):
                    for (w_sb, bd) in ((q_sb, bdq), (k_sb, bdk), (vt_sb, bdv)):
                        pp = qkv_pool.tile([128, 130], f32, tag="qkvps")
                        ncols = 130 if bd is bdk else 128
                        nc.tensor.matmul(
                            pp[:, 0:ncols],
                            lhsT=xT[:, 128 * c:128 * (c + 1)],
                            rhs=bd[:, 0:ncols], start=True, stop=True)
                        if bd is bdv:
                            # psum cols are (m_local, w); scatter into (w, m)
                            pv = pp[:, 0:128].rearrange(
                                "t (m w) -> t w m", m=2)
                            dv = vt_sb[:, :].rearrange(
                                "t (w m) -> t w m", m=H)[:, :, 2 * c:2 * c + 2]
                            nc.scalar.activation(dv, pv, AF.Copy)
                        else:
                            nc.scalar.activation(
                                w_sb[:, 128 * c:128 * (c + 1)], pp[:, 0:128],
                                AF.Copy)
                        if bd is bdk:
                            nc.scalar.activation(
                                r_sb[:, 2 * c:2 * c + 2], pp[:, 128:130],
                                AF.Copy)
                return q_sb, k_sb, vt_sb, r_sb

            def attention(q_sb, k_sb, vt_sb, r_sb):
                """Per-token 16x16 head-mixing attention, all on DVE/ACT."""
                # logits: prod[t,(n,m,w)] = q[t,n,w]*k[t,m,w]  (q pre-scaled)
                prod = bpool.tile([128, H * H * HD], bf16, tag="prod")
                qv = q_sb[:, :].rearrange("t (n o w) -> t n o w", n=H, o=1) \
                    .broadcast_to((128, H, H, HD))
                kv = k_sb[:, :].rearrange("t (o m w) -> t o m w", o=1, m=H) \
                    .broadcast_to((128, H, H, HD))
                p3 = prod[:, :].rearrange("t (n m w) -> t n m w", n=H, m=H)
                nc.vector.tensor_mul(p3, qv, kv)

                pa1 = mpool.tile([128, H * H * HD // 2], bf16, tag="pa1")
                pv_ = prod[:, :].rearrange("t (nm w) -> t nm w", w=HD)
                nc.vector.tensor_add(
                    pa1[:, :].rearrange("t (nm w) -> t nm w", w=HD // 2),
                    pv_[:, :, 0:HD // 2], pv_[:, :, HD // 2:HD])
                pa2 = mpool.tile([128, H * H * HD // 4], bf16, tag="pa2")
                p1v = pa1[:, :].rearrange("t (nm w) -> t nm w", w=HD // 2)
                nc.vector.tensor_add(
                    pa2[:, :].rearrange("t (nm w) -> t nm w", w=HD // 4),
                    p1v[:, :, 0:HD // 4], p1v[:, :, HD // 4:HD // 2])
                pa3 = mpool.tile([128, H * H * HD // 8], bf16, tag="pa3")
                p2v = pa2[:, :].rearrange("t (nm w) -> t nm w", w=HD // 4)
                nc.vector.tensor_add(
                    pa3[:, :].rearrange("t (nm w) -> t nm w", w=HD // 8),
                    p2v[:, :, 0:HD // 8], p2v[:, :, HD // 8:HD // 4])
                pa4 = mpool.tile([128, H * H * HD // 16], bf16, tag="pa4")
                p3v = pa3[:, :].rearrange("t (nm w) -> t nm w", w=HD // 8)
                nc.vector.tensor_add(
                    pa4[:, :].rearrange("t (nm w) -> t nm w", w=HD // 16),
                    p3v[:, :, 0:HD // 16], p3v[:, :, HD // 16:HD // 8])
                # finish the w-reduction with two more 2x-mode adds instead of
                # a 1x-mode TENSOR_REDUCE over [nm, 4]
                pa5 = mpool.tile([128, H * H * HD // 32], bf16, tag="pa5")
                p4v = pa4[:, :].rearrange("t (nm w) -> t nm w", w=HD // 16)
                nc.vector.tensor_add(
                    pa5[:, :].rearrange("t (nm w) -> t nm w", w=HD // 32),
                    p4v[:, :, 0:HD // 32], p4v[:, :, HD // 32:HD // 16])
                L = wpool.tile([128, H * H], f32, tag="L")
                p5v = pa5[:, :].rearrange("t (nm w) -> t nm w", w=HD // 32)
                nc.vector.tensor_add(
                    L[:, :].rearrange("t (nm o) -> t nm o", o=1),
                    p5v[:, :, 0:1], p5v[:, :, 1:2])
                # add r[t,m] broadcast over n; no max-subtraction (|L|
                # is O(8) so f32 exp is safe)
                rv = r_sb[:, :].rearrange("t (o m) -> t o m", o=1) \
                    .broadcast_to((128, H, H))
                L2 = wpool.tile([128, H * H], f32, tag="L2")
                nc.vector.tensor_add(
                    L2[:, :].rearrange("t (n m) -> t n m", n=H),
                    L[:, :].rearrange("t (n m) -> t n m", n=H), rv)

                Ex = wpool.tile([128, H * H], f32, tag="Ex")
                nc.scalar.activation(Ex[:, :], L2[:, :], AF.Exp)

                den = wpool.tile([128, H], f32, tag="den")
                nc.vector.reduce_sum(
                    den[:, :].rearrange("t (n o) -> t n o", o=1),
                    Ex[:, :].rearrange("t (n m) -> t n m", n=H), axis=AX.X)
                dinv = wpool.tile([128, H], f32, tag="dinv")
                nc.vector.reciprocal(dinv[:, :], den[:, :])
                attn_t = wpool.tile([128, H * H], bf16, tag="attn")
                dv_ = dinv[:, :].rearrange("t (n o) -> t n o", o=1) \
                    .broadcast_to((128, H, H))
                nc.vector.tensor_mul(
                    attn_t[:, :].rearrange("t (n m) -> t n m", n=H),
                    Ex[:, :].rearrange("t (n m) -> t n m", n=H), dv_)

                # AV: prod2[t,(n,w,m)] = attn[t,n,m]*vt[t,w,m]; reduce over m.
                # Both operands innermost-contiguous (m) -> DVE 2x mode.
                prod2 = bpool.tile([128, H * HD * H], bf16, tag="prod")
                av = attn_t[:, :].rearrange("t (n o m) -> t n o m", n=H, o=1) \
                    .broadcast_to((128, H, HD, H))
                vv = vt_sb[:, :].rearrange("t (o w m) -> t o w m", o=1, m=H) \
                    .broadcast_to((128, H, HD, H))
                nc.vector.tensor_mul(
                    prod2[:, :].rearrange("t (n w m) -> t n w m", n=H, w=HD),
                    av, vv)
                qa1 = mpool.tile([128, E * H // 2], bf16, tag="pa1")
                q2v = prod2[:, :].rearrange("t (nw m) -> t nw m", m=H)
                nc.vector.tensor_add(
                    qa1[:, :].rearrange("t (nw m) -> t nw m", m=H // 2),
                    q2v[:, :, 0:H // 2], q2v[:, :, H // 2:H])
                qa2 = mpool.tile([128, E * H // 4], bf16, tag="qa2")
                q1v = qa1[:, :].rearrange("t (nw m) -> t nw m", m=H // 2)
                nc.vector.tensor_add(
                    qa2[:, :].rearrange("t (nw m) -> t nw m", m=H // 4),
                    q1v[:, :, 0:H // 4], q1v[:, :, H // 4:H // 2])
                qa3 = mpool.tile([128, E * H // 8], bf16, tag="qa3")
                q2v_ = qa2[:, :].rearrange("t (nw m) -> t nw m", m=H // 4)
                nc.vector.tensor_add(
                    qa3[:, :].rearrange("t (nw m) -> t nw m", m=H // 8),
                    q2v_[:, :, 0:H // 8], q2v_[:, :, H // 8:H // 4])
                vals_bf = wpool.tile([128, E], bf16, tag="valsbf")
                q3v = qa3[:, :].rearrange("t (nw m) -> t nw m", m=H // 8)
                nc.vector.tensor_add(
                    vals_bf[:, :].rearrange("t (nw o) -> t nw o", o=1),
                    q3v[:, :, 0:1], q3v[:, :, 1:2])
                return vals_bf

            def back_end(ti, vals_bf):
                r0 = ti * TOK_TILE
                vT = wpool.tile([128, E], bf16, tag="vT")
                for c in range(N_CHUNK):
                    tp = tps_pool.tile([128, 128], bf16, tag="tps")
                    nc.tensor.transpose(
                        tp[:, :], vals_bf[:, 128 * c:128 * (c + 1)], ident[:, :])
                    nc.scalar.activation(
                        vT[:, 128 * c:128 * (c + 1)], tp[:, :], AF.Copy)

                out_sb = wpool.tile([128, E], f32, tag="outsb")
                for half in range(2):
                    op = o_pool.tile([128, 512], f32, tag="ops")
                    for c in range(N_CHUNK):
                        nc.tensor.matmul(
                            op[:, :],
                            lhsT=vT[:, 128 * c:128 * (c + 1)],
                            rhs=wot[c][:, 512 * half:512 * (half + 1)],
                            start=(c == 0), stop=False)
                    nc.tensor.matmul(
                        op[:, :], lhsT=ones1[:, :],
                        rhs=bo_sb[:, 512 * half:512 * (half + 1)],
                        start=False, stop=True)
                    nc.scalar.activation(
                        out_sb[:, 512 * half:512 * (half + 1)], op[:, :], AF.Copy)
                nc.gpsimd.dma_start(out=out_d[r0:r0 + 128, :], in_=out_sb[:, :])

            # software pipeline: tile ti+1's front-end is emitted before
            # tile ti's attention so PE/ACT prep runs under the DVE chain
            cur = front_end(0)
            for ti in range(N_TILES):
                nxt = front_end(ti + 1) if ti + 1 < N_TILES else None
                vals_bf = attention(*cur)
                back_end(ti, vals_bf)
                cur = nxt
    nc.compile()
    return nc


def _host_consts(wq, bq, wk, bk, wv, bv, wo, bo):
    z = np.zeros((64, 64), np.float32)
    bd2 = lambda w: np.block([[w.T, z], [z, w.T]])
    bdq = (bd2(wq) * SCALE).astype(BF16)
    bdv = bd2(wv).astype(BF16)
    cr = (SCALE * (wk.T @ bq)).astype(np.float32)
    bdk = np.zeros((128, 130), np.float32)
    bdk[:, :128] = bd2(wk)
    bdk[0:64, 128] = cr
    bdk[64:128, 129] = cr
    bdk = bdk.astype(BF16)
    bo_eff = bo + wo @ np.tile(bv, H)
    return dict(
        bdq=bdq, bdk=bdk, bdv=bdv,
        ident=np.eye(128, dtype=BF16),
        wot=np.ascontiguousarray(wo.T).astype(BF16),
        ones1=np.ones((1, 128), BF16),
        bo_eff=bo_eff.reshape(1, E).astype(BF16),
    )


_CACHE = {}


def kernel(x, wq, bq, wk, bk, wv, bv, wo, bo, _trace=False):
    x = np.asarray(x, np.float32)
    # host-side permute e=(w n) -> head-blocked (n w), cast to bf16
    xp = np.ascontiguousarray(
        x.reshape(B, S, HD, H).transpose(0, 1, 3, 2)).reshape(B, S, E)
    xp = xp.astype(BF16)
    consts = _host_consts(
        np.asarray(wq, np.float32), np.asarray(bq, np.float32),
        np.asarray(wk, np.float32), np.asarray(bk, np.float32),
        np.asarray(wv, np.float32), np.asarray(bv, np.float32),
        np.asarray(wo, np.float32), np.asarray(bo, np.float32))
    if "nc" not in _CACHE:
        _CACHE["nc"] = build_nc()
    nc = _CACHE["nc"]
    in_maps = [dict(x=np.ascontiguousarray(xp[i]), **consts) for i in range(B)]
    res = run_bass_kernel_spmd(nc, in_maps, core_ids=list(range(B)), trace=_trace)
    out = np.stack([np.asarray(res.results[i]["out"]) for i in range(B)], axis=0)
    if _trace:
        kernel._last = res
    return out
